# revision 21
# baseline (speedup 1.0000x reference)
"""Trainium2 Bass kernel for a K=1 neighborhood-attention block.

Reference computation (per batch b, N=2048 positions, C=512 channels):
    Q  = x @ Wq^T + bq ;  K = x @ Wk^T + bk ;  V = x @ Wv^T + bv
    s[n]   = Q[n] . K[nbr[n]] + rel_bias[0,0]
    scores = one-hot-sparse [N, N] row n has s[n] at column nbr[n], zeros else
    probs  = softmax(scores / sqrt(C))
    out    = probs @ V[nbr] ;  y = out @ Wo^T + bo

Because each score row is all-zeros except one entry, softmax collapses:
    t[n]  = (s[n]) / sqrt(C); D = e^{t[n]} + (N-1)
    out[n] = (sum_m V[nbr[m]] + (e^{t[n]}-1) * V[nbr[nbr[n]]]) / D
With weight folding A = Wq^T Wk, B = Wv^T Wo^T, beta = Wo bv + bo:
    t[n]  = (x[n] A xg[n]^T + x[n].u + xg[n].v + bq.bk + rb00)/sqrt(C)
    y[n]  = w0[n] * S + w1[n] * P2[n]
      w0 = 1/(e^t + N-1), w1 = 1 - N*w0
      P2[n] = xg2[n] @ B + beta        (xg = x[nbr], xg2 = x[nbr[nbr]])
      S     = sxg @ B + N*beta         (sxg = sum_n xg[n])
So the device does two [2048,512]x[512,512] matmuls per core plus a fused
rowwise dot + exp + a diagonal-matrix epilogue matmul. Data-parallel over
batch: 8 batches over 8 cores, weights replicated.
"""

import math
import os

import numpy as np

# Recover wedged NeuronCores from a previous crashed run at NRT init.
os.environ.setdefault("NEURON_RT_RESET_CORES", "1")

B, N, C = 8, 2048, 512
P = 128
NT = N // P          # 16 n-tiles
KC = C // P          # 4 contraction chunks
FD = 512             # matmul moving free dim / psum bank
INV_SQRT_C = 1.0 / math.sqrt(C)

# matmul-facing dtype: "bfloat16" (1 cyc/row, ~0.5% rel err) or
# "float32r" (1 cyc/row at FD>=256, near-fp32 precision) or "float32" (4x slow)
MM_DT = os.environ.get("NAB_MM_DT", "float32r")
# dtype of the elementwise multiply partner (xg) for the s-dot
XG_DT = os.environ.get("NAB_XG_DT", "float32")

_TRACE = {"enabled": False, "trace_cores": None, "last": None}
_CACHE = {}


def _np_dt(name):
    if name == "bfloat16":
        import ml_dtypes

        return ml_dtypes.bfloat16
    return np.float32


_STAGE = os.environ.get("NAB_STAGE", "full")


def _build_program(mm_dt_str, xg_dt_str, has_beta, has_sbias):
    import concourse.bass as bass
    import concourse.tile as tile
    from concourse import bacc, mybir
    from concourse.bass import ts

    mm_dt = getattr(mybir.dt, mm_dt_str)
    xg_dt = getattr(mybir.dt, xg_dt_str)
    f32 = mybir.dt.float32

    nc = bacc.Bacc("TRN2", target_bir_lowering=False, debug=False)

    # ---- DRAM I/O (per core) ----
    xt_d = nc.dram_tensor("xt", [C, N], mm_dt, kind="ExternalInput")       # x^T
    xg2t_d = nc.dram_tensor("xg2t", [C, N], mm_dt, kind="ExternalInput")   # xg2^T
    xg_d = nc.dram_tensor("xg", [N, C], xg_dt, kind="ExternalInput")       # xg
    a_d = nc.dram_tensor("a", [C, C], mm_dt, kind="ExternalInput")         # A
    bm_d = nc.dram_tensor("bm", [C, C], mm_dt, kind="ExternalInput")       # B
    sxg_d = nc.dram_tensor("sxg", [C, 1], mm_dt, kind="ExternalInput")
    ident_d = nc.dram_tensor("ident", [P, P], mm_dt, kind="ExternalInput")
    ones_d = nc.dram_tensor("ones1", [1, P], mm_dt, kind="ExternalInput")
    if has_beta:
        beta_d = nc.dram_tensor("beta", [1, FD], mm_dt, kind="ExternalInput")
        nbeta_d = nc.dram_tensor("nbeta", [1, FD], mm_dt, kind="ExternalInput")
    if has_sbias:
        sbias_d = nc.dram_tensor("sbias", [P, NT], f32, kind="ExternalInput")
    y_d = nc.dram_tensor("y", [N, C], f32, kind="ExternalOutput")

    with tile.TileContext(nc) as tc:
        with (
            tc.tile_pool(name="singles", bufs=1) as singles,
            tc.tile_pool(name="scratch", bufs=3) as scratch,
            tc.tile_pool(name="diags", bufs=4) as diags,
            tc.tile_pool(name="mm_psum", bufs=2, space="PSUM") as mm_psum,
            tc.tile_pool(name="misc_psum", bufs=1, space="PSUM") as misc_psum,
            tc.tile_pool(name="out_psum", bufs=2, space="PSUM") as out_psum_pool,
        ):
            # ---- persistent SBUF ----
            xt_sb = singles.tile([P, KC, N], mm_dt)
            xg2t_sb = singles.tile([P, KC, N], mm_dt)
            xg_sb = singles.tile([P, NT, C], xg_dt)
            a_sb = singles.tile([P, KC, C], mm_dt)
            bm_sb = singles.tile([P, KC, C], mm_dt)
            sxg_sb = singles.tile([P, KC, 1], mm_dt)
            ident_sb = singles.tile([P, P], mm_dt)
            ones_sb = singles.tile([1, P], mm_dt)
            p2_all = singles.tile([P, NT, FD], mm_dt)
            s2row_sb = singles.tile([1, FD], mm_dt)
            s2b_sb = singles.tile([P, FD], mm_dt)
            s_all = singles.tile([P, NT], f32)
            e_all = singles.tile([P, NT], f32)
            w0_all = singles.tile([P, NT], f32)
            w1_all = singles.tile([P, NT], f32)

            # ---- constant / weight loads ----
            nc.sync.dma_start(a_sb[:], a_d.ap().rearrange("(kc p) c -> p kc c", p=P))
            nc.sync.dma_start(bm_sb[:], bm_d.ap().rearrange("(kc p) c -> p kc c", p=P))
            nc.sync.dma_start(sxg_sb[:], sxg_d.ap().rearrange("(kc p) o -> p kc o", p=P))
            nc.sync.dma_start(ident_sb[:], ident_d[:])
            nc.sync.dma_start(ones_sb[:], ones_d[:])
            if has_beta:
                beta_sb = singles.tile([1, FD], mm_dt)
                nbeta_sb = singles.tile([1, FD], mm_dt)
                nc.sync.dma_start(beta_sb[:], beta_d[:])
                nc.sync.dma_start(nbeta_sb[:], nbeta_d[:])
            if has_sbias:
                sbias_sb = singles.tile([P, NT], f32)
                nc.sync.dma_start(sbias_sb[:], sbias_d[:])

            xt_ap = xt_d.ap().rearrange("(kc p) n -> p kc n", p=P)
            xg2t_ap = xg2t_d.ap().rearrange("(kc p) n -> p kc n", p=P)
            xg_ap = xg_d.ap().rearrange("(nt p) c -> p nt c", p=P)

            do_s2 = _STAGE in ("a", "full")
            do_ttr = _STAGE in ("b2", "b3", "a", "full")
            do_p2 = _STAGE in ("b3", "b4", "a", "full")
            do_xa = _STAGE in ("b", "b2", "b3", "b4", "a", "full")

            # ---- S'' = sxg @ B (+ N*beta), then broadcast to 128 partitions
            if do_s2:
                s2_psum = misc_psum.tile([1, FD], f32, tag="s2")
                for kc in range(KC):
                    nc.tensor.matmul(
                        s2_psum[:],
                        sxg_sb[:, kc, :],
                        bm_sb[:, kc, :],
                        start=(kc == 0),
                        stop=(kc == KC - 1 and not has_beta),
                    )
                if has_beta:
                    nc.tensor.matmul(
                        s2_psum[:], ones_sb[:, 0:1], nbeta_sb[:], start=False, stop=True
                    )
                nc.scalar.copy(out=s2row_sb[:], in_=s2_psum[:])
                s2b_psum = misc_psum.tile([P, FD], f32, tag="s2b")
                nc.tensor.matmul(s2b_psum[:], ones_sb[:], s2row_sb[:], start=True, stop=True)
                nc.scalar.copy(out=s2b_sb[:], in_=s2b_psum[:])

            # ---- main per-tile pipeline ----
            for g in range(4):  # 4 groups of 4 n-tiles; DMA chunks interleaved
                nsl = slice(g * 512, (g + 1) * 512)
                nc.sync.dma_start(xt_sb[:, :, nsl], xt_ap[:, :, nsl])
                nc.sync.dma_start(xg2t_sb[:, :, nsl], xg2t_ap[:, :, nsl])
                nc.sync.dma_start(xg_sb[:, 4 * g : 4 * g + 4, :], xg_ap[:, 4 * g : 4 * g + 4, :])
                for ti in range(4 * g, 4 * g + 4):
                    # XA = (x @ A) for this n-tile; then s = rowdot(XA, xg)
                    if do_xa:
                        xa_psum = mm_psum.tile([P, FD], f32, tag="xa")
                        for kc in range(KC):
                            nc.tensor.matmul(
                                xa_psum[:],
                                xt_sb[:, kc, ts(ti, P)],
                                a_sb[:, kc, :],
                                start=(kc == 0),
                                stop=(kc == KC - 1),
                            )
                    if do_ttr:
                        prod = scratch.tile([P, FD], f32, tag="prod")
                        nc.vector.tensor_tensor(
                            prod[:], xa_psum[:], xg_sb[:, ti, :], mybir.AluOpType.mult
                        )
                        nc.vector.tensor_reduce(
                            out=s_all[:, ti : ti + 1],
                            in_=prod[:],
                            axis=mybir.AxisListType.X,
                            op=mybir.AluOpType.add,
                        )
                    elif do_xa and not do_p2:
                        nc.vector.tensor_copy(p2_all[:, ti, :], xa_psum[:])
                    if do_p2:
                        # P2 = xg2 @ B (+ beta)
                        p2_psum = mm_psum.tile([P, FD], f32, tag="p2")
                        for kc in range(KC):
                            nc.tensor.matmul(
                                p2_psum[:],
                                xg2t_sb[:, kc, ts(ti, P)],
                                bm_sb[:, kc, :],
                                start=(kc == 0),
                                stop=(kc == KC - 1 and not has_beta),
                            )
                        if has_beta:
                            nc.tensor.matmul(
                                p2_psum[:], ones_sb[:, 0:1], beta_sb[:], start=False, stop=True
                            )
                        nc.scalar.copy(out=p2_all[:, ti, :], in_=p2_psum[:])
                    elif not do_xa:
                        nc.vector.tensor_copy(
                            p2_all[:, ti, :], xg_sb[:, ti, :]
                        )

            if _STAGE in ("a", "b", "b2", "b3", "b4", "c"):
                # bisect stages: skip exp/w/epilogue; dump p2_all tiles to y
                for ti in range(NT):
                    o_sb = scratch.tile([P, FD], f32, tag="osb")
                    nc.vector.tensor_copy(o_sb[:], p2_all[:, ti, :])
                    nc.sync.dma_start(y_d[ts(ti, P), :], o_sb[:])
            else:
                # ---- softmax weights: e = exp(t/sqrt(C)); w0 = 1/(e+N-1); w1 = 1 - N*w0
                if has_sbias:
                    nc.vector.tensor_tensor(
                        s_all[:], s_all[:], sbias_sb[:], mybir.AluOpType.add
                    )
                nc.scalar.activation(
                    out=e_all[:],
                    in_=s_all[:],
                    func=mybir.ActivationFunctionType.Exp,
                    scale=INV_SQRT_C,
                )
                nc.vector.tensor_scalar_add(w1_all[:], e_all[:], float(N - 1))
                nc.vector.reciprocal(w0_all[:], w1_all[:])
                nc.vector.tensor_scalar(
                    out=w1_all[:],
                    in0=w0_all[:],
                    scalar1=float(-N),
                    scalar2=1.0,
                    op0=mybir.AluOpType.mult,
                    op1=mybir.AluOpType.add,
                )

                # ---- epilogue: y[n] = w0[n]*S'' + w1[n]*P2[n] via diag matmuls
                for ti in range(NT):
                    diag1 = diags.tile([P, P], mm_dt, tag="diag1")
                    diag0 = diags.tile([P, P], mm_dt, tag="diag0")
                    nc.vector.tensor_scalar_mul(diag1[:], ident_sb[:], w1_all[:, ti : ti + 1])
                    nc.vector.tensor_scalar_mul(diag0[:], ident_sb[:], w0_all[:, ti : ti + 1])
                    o_psum = out_psum_pool.tile([P, FD], f32, tag="out")
                    nc.tensor.matmul(o_psum[:], diag1[:], p2_all[:, ti, :], start=True, stop=False)
                    nc.tensor.matmul(o_psum[:], diag0[:], s2b_sb[:], start=False, stop=True)
                    o_sb = scratch.tile([P, FD], f32, tag="osb")
                    nc.vector.tensor_copy(o_sb[:], o_psum[:])
                    nc.sync.dma_start(y_d[ts(ti, P), :], o_sb[:])

    nc.compile()
    return nc


def kernel(x, neighbors, Wq, bq, Wk, bk, Wv, bv, rel_bias, Wo, bo):
    from concourse.bass_utils import run_bass_kernel_spmd

    x = np.asarray(x, dtype=np.float32)
    Wq = np.asarray(Wq, dtype=np.float32)
    Wk = np.asarray(Wk, dtype=np.float32)
    Wv = np.asarray(Wv, dtype=np.float32)
    Wo = np.asarray(Wo, dtype=np.float32)
    bq = np.asarray(bq, dtype=np.float32)
    bk = np.asarray(bk, dtype=np.float32)
    bv = np.asarray(bv, dtype=np.float32)
    bo = np.asarray(bo, dtype=np.float32)
    rel_bias = np.asarray(rel_bias, dtype=np.float32)
    nbr = np.asarray(neighbors).reshape(N, -1)[:, 0].astype(np.int64)
    nbr2 = nbr[nbr]

    mm_np = _np_dt(MM_DT)
    xg_np = _np_dt(XG_DT)

    # host-side weight folding (tiny)
    A = (Wq.T @ Wk).astype(np.float32)            # [C, C]
    Bm = (Wv.T @ Wo.T).astype(np.float32)         # [C, C]
    beta = (Wo @ bv + bo).astype(np.float32)      # [C]
    u = (Wq.T @ bk).astype(np.float32)
    v = (Wk.T @ bq).astype(np.float32)
    const = float(bq @ bk) + float(rel_bias[0, 0])

    xg = x[:, nbr, :]                             # [B, N, C]
    xg2 = x[:, nbr2, :]
    sxg = xg.sum(axis=1)                          # [B, C]
    # raw (pre-1/sqrt(C)) additive score bias; the scale is applied inside exp
    sbias = x @ u + xg @ v + const  # [B, N]

    has_beta = bool(np.any(beta != 0.0))
    has_sbias = bool(np.any(sbias != 0.0))

    key = (MM_DT, XG_DT, has_beta, has_sbias)
    if key not in _CACHE:
        _CACHE[key] = _build_program(*key)
    nc = _CACHE[key]

    ident = np.eye(P, dtype=mm_np)
    ones1 = np.ones((1, P), dtype=mm_np)
    in_maps = []
    for b in range(B):
        m = {
            "xt": np.ascontiguousarray(x[b].T).astype(mm_np),
            "xg2t": np.ascontiguousarray(xg2[b].T).astype(mm_np),
            "xg": np.ascontiguousarray(xg[b]).astype(xg_np),
            "a": A.astype(mm_np),
            "bm": Bm.astype(mm_np),
            "sxg": np.ascontiguousarray(sxg[b][:, None]).astype(mm_np),
            "ident": ident,
            "ones1": ones1,
        }
        if has_beta:
            m["beta"] = beta[None, :].astype(mm_np)
            m["nbeta"] = (float(N) * beta)[None, :].astype(mm_np)
        if has_sbias:
            m["sbias"] = np.ascontiguousarray(sbias[b].reshape(NT, P).T).astype(
                np.float32
            )
        in_maps.append(m)

    res = run_bass_kernel_spmd(
        nc,
        in_maps,
        core_ids=list(range(B)),
        trace=_TRACE["enabled"],
        trace_cores=_TRACE["trace_cores"],
    )
    _TRACE["last"] = res
    y = np.stack([r["y"] for r in res.results], axis=0)
    return y.astype(np.float32)
